# revision 35
# baseline (speedup 1.0000x reference)
"""Single-head causal attention (nanoGPT Head) on 8 TRN2 NeuronCores.

Sharding: data-parallel over batch. B=8 batch elements -> one per core.
Each core computes, for its x_b [T=2048, E=1024] and shared Wq/Wk/Wv [E, H=128]:
    q = x @ Wq ; k = x @ Wk ; v = x @ Wv
    out = softmax(causal(q k^T / sqrt(H))) v          -> [T, H]

Per-core pipeline (all matmuls contract along the SBUF partition dim):
  1. DMA x tiles [128, E]; PE-transpose (f32) into xT [E-part, T] rounded to
     f32r during the mandatory PSUM->SBUF copy (f32r streams at 1 cyc/row).
  2. qT/kT = W^T x^T via f32r matmuls (N=512 chunks, 8 e-tile accumulation);
     vT likewise, evacuated as bf16 and re-transposed to V [kv-part, H].
  3. Per q-tile (128 queries): S chunks [128, <=512] = qT_tile^T kT (f32r);
     causal tri-mask added on the diagonal 128-col block in PSUM; ACT Exp
     with accum_out produces P (bf16) + exact row sums l. No max-subtraction:
     scores are ~N(0,1), bounded ~ +-6, exp is safe in f32.
  4. PE-transpose P tiles (bf16) -> PT; PV matmuls (bf16, N=128) accumulate
     out [q, H] in PSUM over kv tiles; multiply by 1/l (per-partition) on the
     PSUM->SBUF copy; DMA out.
"""
import numpy as np

import concourse.bacc as bacc
import concourse.mybir as mybir
import concourse.tile as tile
from concourse.bass_utils import run_bass_kernel_spmd
from concourse.masks import make_identity, make_causal_mask

FP32 = mybir.dt.float32
FP32R = mybir.dt.float32r
BF16 = mybir.dt.bfloat16
AF = mybir.ActivationFunctionType

T = 2048          # sequence length (per core)
E = 1024          # embedding dim
H = 128           # head size
NT = T // 128     # 16 query/kv tiles
NE = E // 128     # 8 embedding tiles
SCALE = 1.0 / float(np.sqrt(H))
MASK_VAL = -1e9


def build():
    nc = bacc.Bacc()
    x_ext = nc.declare_dram_parameter("x", [T, E], FP32, isOutput=False)
    wq_ext = nc.declare_dram_parameter("Wq", [E, H], FP32, isOutput=False)
    wk_ext = nc.declare_dram_parameter("Wk", [E, H], FP32, isOutput=False)
    wv_ext = nc.declare_dram_parameter("Wv", [E, H], FP32, isOutput=False)
    out_ext = nc.declare_dram_parameter("out", [T, H], FP32, isOutput=True)

    with tile.TileContext(nc) as tc:
        with (
            tc.tile_pool(name="const", bufs=1) as const,
            tc.tile_pool(name="big", bufs=1) as big,
            tc.tile_pool(name="xstage", bufs=3) as xstage,
            tc.tile_pool(name="pbuf", bufs=3) as pbuf,
            tc.tile_pool(name="ptbuf", bufs=3) as ptbuf,
            tc.tile_pool(name="small", bufs=3) as small,
            tc.tile_pool(name="ps_t", bufs=2, space="PSUM") as ps_t_pool,
            tc.tile_pool(name="ps_proj", bufs=2, space="PSUM") as ps_proj_pool,
            tc.tile_pool(name="ps_s", bufs=3, space="PSUM") as ps_s_pool,
            tc.tile_pool(name="ps_l", bufs=1, space="PSUM") as ps_l_pool,
        ):
            # ---- constants (built on-chip, no DMA waits) ----
            ident = const.tile([128, 128], FP32, tag="ident")
            identb = const.tile([128, 128], BF16, tag="identb")
            mask_tri = const.tile([128, 128], FP32, tag="mask")
            make_identity(nc, ident[:])
            make_identity(nc, identb[:])
            make_causal_mask(nc, mask_tri[:], mask_val=MASK_VAL)

            # ---- weights: DMA f32, round to f32r ----
            w_r = []
            for name, ext in (("wq", wq_ext), ("wk", wk_ext), ("wv", wv_ext)):
                w_f = const.tile([128, E], FP32, tag=f"{name}f")
                # W[(k p) h] -> sbuf[p, (k h)]
                nc.gpsimd.dma_start(
                    w_f[:].rearrange("p (k h) -> p k h", k=NE),
                    ext[:].rearrange("(k p) h -> p k h", p=128))
                w_rr = const.tile([128, E], FP32R, tag=f"{name}r")
                nc.gpsimd.tensor_copy(w_rr[:], w_f[:])
                w_r.append(w_rr)
            wq_r, wk_r, wv_r = w_r

            # ---- persistent big buffers ----
            xT = big.tile([128, NE * T], FP32R, tag="xT")       # [e-part, k*T + t]
            qT = big.tile([128, T], FP32R, tag="qT")            # [h, t]
            kT = big.tile([128, T], FP32R, tag="kT")            # [h, t]
            vT = big.tile([128, T], BF16, tag="vT")             # [h, t]
            V = big.tile([128, T], BF16, tag="V")               # [kv-part, j*H + h]

            # ---- phase 1: x -> xT (PE transpose, 2 batched copies per tile) ----
            for i in range(NT):
                x_t = xstage.tile([128, E], FP32, tag="xs")
                nc.sync.dma_start(x_t[:], x_ext[128 * i:128 * (i + 1), :])
                for g in range(2):                       # groups of 4 e-tiles
                    ps4 = ps_t_pool.tile([128, 512], FP32, tag="pst")
                    for kk in range(4):
                        k = 4 * g + kk
                        nc.tensor.transpose(
                            ps4[:, 128 * kk:128 * (kk + 1)],
                            x_t[:, 128 * k:128 * (k + 1)], ident[:])
                    # scatter the 4 transposed blocks to their e-tile columns
                    dst = xT[:].rearrange("p (k t) -> p k t", k=NE)[
                        :, 4 * g:4 * (g + 1), 128 * i:128 * (i + 1)]
                    eng = nc.vector if (i + g) % 2 == 0 else nc.scalar
                    if eng is nc.vector:
                        nc.vector.tensor_copy(dst, ps4[:].rearrange("p (k t) -> p k t", k=4))
                    else:
                        nc.scalar.copy(dst, ps4[:].rearrange("p (k t) -> p k t", k=4))

            # ---- phase 2: projections qT/kT/vT (f32r, N=512) ----
            for c in range(T // 512):
                sl = slice(512 * c, 512 * (c + 1))
                for pi, (w, dstT) in enumerate(((wq_r, qT), (wk_r, kT), (wv_r, vT))):
                    psp = ps_proj_pool.tile([128, 512], FP32, tag="psp")
                    for k in range(NE):
                        nc.tensor.matmul(
                            psp[:], w[:, 128 * k:128 * (k + 1)],
                            xT[:, k * T + 512 * c:k * T + 512 * (c + 1)],
                            start=(k == 0), stop=(k == NE - 1))
                    if pi == 0:
                        nc.scalar.copy(dstT[:, sl], psp[:])
                    elif pi == 1:
                        nc.vector.tensor_copy(dstT[:, sl], psp[:])
                    else:
                        nc.vector.tensor_copy(dstT[:, sl], psp[:])  # vT as bf16

            # ---- phase 3: V = vT^T  (bf16 transposes) ----
            for g in range(4):
                ps4 = ps_t_pool.tile([128, 512], BF16, tag="pst")
                for jj in range(4):
                    j = 4 * g + jj
                    nc.tensor.transpose(
                        ps4[:, 128 * jj:128 * (jj + 1)],
                        vT[:, 128 * j:128 * (j + 1)], identb[:])
                eng = nc.vector if g % 2 == 0 else nc.scalar
                if eng is nc.vector:
                    nc.vector.tensor_copy(V[:, 512 * g:512 * (g + 1)], ps4[:])
                else:
                    nc.scalar.copy(V[:, 512 * g:512 * (g + 1)], ps4[:])

            # ---- phase 4: attention in transposed-score form.
            # Per q-chunk c (512 queries), per kv-tile t<=4c+3:
            #   S^T[kv,q] = kT_tile^T qT_chunk (f32r, N=512); causal mask on
            #   diagonal tiles; ACT Exp -> P^T bf16 (no PE transposes of P);
            #   l row via ones-matmul (M=1); out^T[h,q] += V_t^T P^T (N=512).
            # Finalize: recip row -> PE outer-product broadcast -> normalize
            # in [h,q], PE-transpose to [q,h], DMA out.
            ones_bf = const.tile([128, 1], BF16, tag="ones")
            nc.gpsimd.memset(ones_bf[:], 1.0)
            ones_row = const.tile([1, 128], FP32, tag="onesr")
            nc.gpsimd.memset(ones_row[:], 1.0)
            mask_triT = const.tile([128, 128], FP32, tag="maskT")
            nc.gpsimd.memset(mask_triT[:], 0.0)
            nc.gpsimd.affine_select(
                out=mask_triT[:], in_=mask_triT[:],
                compare_op=mybir.AluOpType.is_ge, fill=MASK_VAL, base=0,
                # keep (0.0) where j - i >= 0 i.e. q >= kv; fill below diagonal
                pattern=[[1, 128]], channel_multiplier=-1)

            for c in range(3, -1, -1):                 # largest q-chunk first
                ncv = 4 * c + 4                        # kv tiles in play
                out_ps = ps_proj_pool.tile([128, 512], FP32, tag="psp")
                l_ps = ps_l_pool.tile([128, 512], FP32, tag="psl")
                for t in range(ncv):
                    pss = ps_s_pool.tile([128, 512], FP32, tag="pss")
                    nc.tensor.matmul(
                        pss[:], kT[:, 128 * t:128 * (t + 1)],
                        qT[:, 512 * c:512 * (c + 1)],
                        start=True, stop=True)
                    d = t - 4 * c                      # diag block index if >=0
                    off = 0
                    if d >= 0:
                        nc.vector.tensor_add(
                            pss[:, 128 * d:128 * (d + 1)],
                            pss[:, 128 * d:128 * (d + 1)], mask_triT[:])
                        off = 128 * d                  # cols < off fully invalid
                    pb = ptbuf.tile([128, 512], BF16, tag="pt")
                    if off > 0:
                        nc.gpsimd.memset(pb[:, :off], 0.0)
                    nc.scalar.activation(
                        pb[:, off:], pss[:, off:], AF.Exp, bias=0.0, scale=SCALE)
                    nc.tensor.matmul(
                        l_ps[0:1, :], ones_bf[:], pb[:],
                        start=(t == 0), stop=(t == ncv - 1))
                    nc.tensor.matmul(
                        out_ps[:], V[:, 128 * t:128 * (t + 1)], pb[:],
                        start=(t == 0), stop=(t == ncv - 1))

                # finalize: 1/l broadcast (PE outer product), normalize, transpose
                l_row = small.tile([128, 512], FP32, tag="lrow")
                r_row = small.tile([128, 512], FP32, tag="rrow")
                nc.vector.tensor_copy(l_row[0:1, :], l_ps[0:1, :])
                nc.vector.reciprocal(r_row[0:1, :], l_row[0:1, :])
                rb_ps = ps_proj_pool.tile([128, 512], FP32, tag="psp")
                nc.tensor.matmul(rb_ps[:], ones_row[:], r_row[0:1, :], start=True, stop=True)
                rb_sb = small.tile([128, 512], BF16, tag="rb")
                nc.scalar.copy(rb_sb[:], rb_ps[:])
                oT_sb = small.tile([128, 512], BF16, tag="ot")
                nc.vector.tensor_copy(oT_sb[:], out_ps[:])
                nm_sb = small.tile([128, 512], BF16, tag="nm")
                nc.vector.tensor_mul(nm_sb[:], oT_sb[:], rb_sb[:])
                ps4 = ps_t_pool.tile([128, 512], BF16, tag="pst")
                for j in range(4):
                    nc.tensor.transpose(
                        ps4[:, 128 * j:128 * (j + 1)],
                        nm_sb[:, 128 * j:128 * (j + 1)], identb[:])
                ostage = small.tile([128, 512], FP32, tag="os")
                nc.scalar.copy(ostage[:], ps4[:])
                for j in range(4):
                    nc.sync.dma_start(
                        out_ext[512 * c + 128 * j:512 * c + 128 * (j + 1), :],
                        ostage[:, 128 * j:128 * (j + 1)])

    nc.compile()
    return nc


_NC_CACHE = None


def _get_nc():
    global _NC_CACHE
    if _NC_CACHE is None:
        _NC_CACHE = build()
    return _NC_CACHE


def kernel(x, Wq, Wk, Wv):
    """x: [8, 2048, 1024] f32; Wq/Wk/Wv: [1024, 128] f32 -> [8, 2048, 128] f32."""
    x = np.ascontiguousarray(x, dtype=np.float32)
    Wq = np.ascontiguousarray(Wq, dtype=np.float32)
    Wk = np.ascontiguousarray(Wk, dtype=np.float32)
    Wv = np.ascontiguousarray(Wv, dtype=np.float32)
    B = x.shape[0]
    assert x.shape == (B, T, E) and B == 8

    nc = _get_nc()
    in_maps = [{"x": x[b], "Wq": Wq, "Wk": Wk, "Wv": Wv} for b in range(B)]
    res = run_bass_kernel_spmd(nc, in_maps, core_ids=list(range(B)))
    return np.stack([res.results[b]["out"] for b in range(B)], axis=0)


if __name__ == "__main__":
    rng = np.random.default_rng(0)
    x = rng.standard_normal((8, T, E), dtype=np.float32)
    s = 1.0 / np.sqrt(E)
    Wq = (rng.standard_normal((E, H)) * s).astype(np.float32)
    Wk = (rng.standard_normal((E, H)) * s).astype(np.float32)
    Wv = (rng.standard_normal((E, H)) * s).astype(np.float32)
    out = kernel(x=x, Wq=Wq, Wk=Wk, Wv=Wv)
    print("out", out.shape, out.dtype, np.abs(out).max())


# revision 37
# speedup vs baseline: 1.1790x; 1.1790x over previous
"""Single-head causal attention (nanoGPT Head) on 8 TRN2 NeuronCores.

Sharding: data-parallel over batch. B=8 batch elements -> one per core.
Each core computes, for its x_b [T=2048, E=1024] and shared Wq/Wk/Wv [E, H=128]:
    q = x @ Wq ; k = x @ Wk ; v = x @ Wv
    out = softmax(causal(q k^T / sqrt(H))) v          -> [T, H]

Per-core pipeline (all matmuls contract along the SBUF partition dim):
  1. DMA x tiles [128, E]; PE-transpose (f32) into xT [E-part, T] rounded to
     f32r during the mandatory PSUM->SBUF copy (f32r streams at 1 cyc/row).
  2. qT/kT = W^T x^T via f32r matmuls (N=512 chunks, 8 e-tile accumulation);
     vT likewise, evacuated as bf16 and re-transposed to V [kv-part, H].
  3. Per q-tile (128 queries): S chunks [128, <=512] = qT_tile^T kT (f32r);
     causal tri-mask added on the diagonal 128-col block in PSUM; ACT Exp
     with accum_out produces P (bf16) + exact row sums l. No max-subtraction:
     scores are ~N(0,1), bounded ~ +-6, exp is safe in f32.
  4. PE-transpose P tiles (bf16) -> PT; PV matmuls (bf16, N=128) accumulate
     out [q, H] in PSUM over kv tiles; multiply by 1/l (per-partition) on the
     PSUM->SBUF copy; DMA out.
"""
import numpy as np

import concourse.bacc as bacc
import concourse.mybir as mybir
import concourse.tile as tile
from concourse.bass_utils import run_bass_kernel_spmd
from concourse.masks import make_identity, make_causal_mask

FP32 = mybir.dt.float32
FP32R = mybir.dt.float32r
BF16 = mybir.dt.bfloat16
AF = mybir.ActivationFunctionType

T = 2048          # sequence length (per core)
E = 1024          # embedding dim
H = 128           # head size
NT = T // 128     # 16 query/kv tiles
NE = E // 128     # 8 embedding tiles
SCALE = 1.0 / float(np.sqrt(H))
MASK_VAL = -1e9


def build():
    nc = bacc.Bacc()
    x_ext = nc.declare_dram_parameter("x", [T, E], FP32, isOutput=False)
    wq_ext = nc.declare_dram_parameter("Wq", [E, H], FP32, isOutput=False)
    wk_ext = nc.declare_dram_parameter("Wk", [E, H], FP32, isOutput=False)
    wv_ext = nc.declare_dram_parameter("Wv", [E, H], FP32, isOutput=False)
    out_ext = nc.declare_dram_parameter("out", [T, H], FP32, isOutput=True)

    with tile.TileContext(nc) as tc:
        with (
            tc.tile_pool(name="const", bufs=1) as const,
            tc.tile_pool(name="big", bufs=1) as big,
            tc.tile_pool(name="xstage", bufs=3) as xstage,
            tc.tile_pool(name="pbuf", bufs=3) as pbuf,
            tc.tile_pool(name="ptbuf", bufs=3) as ptbuf,
            tc.tile_pool(name="small", bufs=3) as small,
            tc.tile_pool(name="ps_t", bufs=2, space="PSUM") as ps_t_pool,
            tc.tile_pool(name="ps_proj", bufs=2, space="PSUM") as ps_proj_pool,
            tc.tile_pool(name="ps_s", bufs=3, space="PSUM") as ps_s_pool,
            tc.tile_pool(name="ps_l", bufs=1, space="PSUM") as ps_l_pool,
        ):
            # ---- constants (built on-chip, no DMA waits) ----
            ident = const.tile([128, 128], FP32, tag="ident")
            identb = const.tile([128, 128], BF16, tag="identb")
            mask_tri = const.tile([128, 128], FP32, tag="mask")
            make_identity(nc, ident[:])
            make_identity(nc, identb[:])
            make_causal_mask(nc, mask_tri[:], mask_val=MASK_VAL)

            # ---- weights: DMA f32, round to f32r ----
            w_r = []
            for name, ext in (("wq", wq_ext), ("wk", wk_ext), ("wv", wv_ext)):
                w_f = const.tile([128, E], FP32, tag=f"{name}f")
                # W[(k p) h] -> sbuf[p, (k h)]
                nc.gpsimd.dma_start(
                    w_f[:].rearrange("p (k h) -> p k h", k=NE),
                    ext[:].rearrange("(k p) h -> p k h", p=128))
                w_rr = const.tile([128, E], FP32R, tag=f"{name}r")
                nc.gpsimd.tensor_copy(w_rr[:], w_f[:])
                w_r.append(w_rr)
            wq_r, wk_r, wv_r = w_r

            # ---- persistent big buffers ----
            xT = big.tile([128, NE * T], FP32R, tag="xT")       # [e-part, k*T + t]
            qT = big.tile([128, T], FP32R, tag="qT")            # [h, t]
            kT = big.tile([128, T], FP32R, tag="kT")            # [h, t]
            vT = big.tile([128, T], BF16, tag="vT")             # [h, t]
            V = big.tile([128, T], BF16, tag="V")               # [kv-part, j*H + h]

            # ---- phase 1: x -> xT (PE transpose, 2 batched copies per tile) ----
            for i in range(NT):
                x_t = xstage.tile([128, E], FP32, tag="xs")
                nc.sync.dma_start(x_t[:], x_ext[128 * i:128 * (i + 1), :])
                for g in range(2):                       # groups of 4 e-tiles
                    ps4 = ps_t_pool.tile([128, 512], FP32, tag="pst")
                    for kk in range(4):
                        k = 4 * g + kk
                        nc.tensor.transpose(
                            ps4[:, 128 * kk:128 * (kk + 1)],
                            x_t[:, 128 * k:128 * (k + 1)], ident[:])
                    # scatter the 4 transposed blocks to their e-tile columns
                    dst = xT[:].rearrange("p (k t) -> p k t", k=NE)[
                        :, 4 * g:4 * (g + 1), 128 * i:128 * (i + 1)]
                    eng = nc.vector if (i + g) % 2 == 0 else nc.scalar
                    if eng is nc.vector:
                        nc.vector.tensor_copy(dst, ps4[:].rearrange("p (k t) -> p k t", k=4))
                    else:
                        nc.scalar.copy(dst, ps4[:].rearrange("p (k t) -> p k t", k=4))

            # ---- phase 2: projections qT/kT/vT (f32r, N=512) ----
            for c in range(T // 512):
                sl = slice(512 * c, 512 * (c + 1))
                for pi, (w, dstT) in enumerate(((wq_r, qT), (wk_r, kT), (wv_r, vT))):
                    psp = ps_proj_pool.tile([128, 512], FP32, tag="psp")
                    for k in range(NE):
                        nc.tensor.matmul(
                            psp[:], w[:, 128 * k:128 * (k + 1)],
                            xT[:, k * T + 512 * c:k * T + 512 * (c + 1)],
                            start=(k == 0), stop=(k == NE - 1))
                    if pi == 0:
                        nc.scalar.copy(dstT[:, sl], psp[:])
                    elif pi == 1:
                        nc.vector.tensor_copy(dstT[:, sl], psp[:])
                    else:
                        nc.vector.tensor_copy(dstT[:, sl], psp[:])  # vT as bf16

            # ---- phase 3: V = vT^T  (bf16 transposes) ----
            for g in range(4):
                ps4 = ps_t_pool.tile([128, 512], BF16, tag="pst")
                for jj in range(4):
                    j = 4 * g + jj
                    nc.tensor.transpose(
                        ps4[:, 128 * jj:128 * (jj + 1)],
                        vT[:, 128 * j:128 * (j + 1)], identb[:])
                eng = nc.vector if g % 2 == 0 else nc.scalar
                if eng is nc.vector:
                    nc.vector.tensor_copy(V[:, 512 * g:512 * (g + 1)], ps4[:])
                else:
                    nc.scalar.copy(V[:, 512 * g:512 * (g + 1)], ps4[:])

            # ---- phase 4: transposed-score attention, software-pipelined.
            # Per q-chunk c (512 queries), kv-tile t: S^T[kv,q] = kT_t^T qT_c
            # (f32r N=512); mask on diagonal tiles; ACT Exp -> P^T bf16 (no P
            # transposes); l row via ones-matmul (M=1); out^T += V_t^T P^T
            # (N=512). exp(t) overlaps S(t+1) + l/PV(t-1) on the in-order PE
            # queue. Finalize: 1/l broadcast via K=1 outer product, normalize
            # in [h,q], PE-transpose to [q,h], DMA out.
            ones_bf = const.tile([128, 1], BF16, tag="ones")
            nc.gpsimd.memset(ones_bf[:], 1.0)
            ones_row = const.tile([1, 128], FP32, tag="onesr")
            nc.gpsimd.memset(ones_row[:], 1.0)
            mask_triT = const.tile([128, 128], FP32, tag="maskT")
            nc.gpsimd.memset(mask_triT[:], 0.0)
            nc.gpsimd.affine_select(
                out=mask_triT[:], in_=mask_triT[:],
                compare_op=mybir.AluOpType.is_ge, fill=MASK_VAL, base=0,
                # keep 0.0 where q - kv >= 0; fill below the diagonal
                pattern=[[1, 128]], channel_multiplier=-1)

            for c in range(3, -1, -1):                 # largest q-chunk first
                ncv = 4 * c + 4
                out_ps = ps_proj_pool.tile([128, 512], FP32, tag="psp")
                l_ps = ps_l_pool.tile([128, 512], FP32, tag="psl")
                pbs = {}

                def s_exp(t):
                    pss = ps_s_pool.tile([128, 512], FP32, tag="pss")
                    nc.tensor.matmul(
                        pss[:], kT[:, 128 * t:128 * (t + 1)],
                        qT[:, 512 * c:512 * (c + 1)], start=True, stop=True)
                    d = t - 4 * c
                    off = 0
                    if d >= 0:                         # diagonal tile
                        nc.vector.tensor_add(
                            pss[:, 128 * d:128 * (d + 1)],
                            pss[:, 128 * d:128 * (d + 1)], mask_triT[:])
                        off = 128 * d                  # cols < off fully invalid
                    pb = ptbuf.tile([128, 512], BF16, tag="pt")
                    if off > 0:
                        nc.gpsimd.memset(pb[:, :off], 0.0)
                    nc.scalar.activation(
                        pb[:, off:], pss[:, off:], AF.Exp, bias=0.0, scale=SCALE)
                    pbs[t] = pb

                def acc(t):
                    pb = pbs.pop(t)
                    nc.tensor.matmul(l_ps[0:1, :], ones_bf[:], pb[:],
                                     start=(t == 0), stop=(t == ncv - 1))
                    nc.tensor.matmul(out_ps[:], V[:, 128 * t:128 * (t + 1)], pb[:],
                                     start=(t == 0), stop=(t == ncv - 1))

                for t in range(ncv):
                    s_exp(t)
                    if t >= 1:
                        acc(t - 1)
                acc(ncv - 1)

                # finalize: 1/l broadcast (PE outer product), normalize, transpose
                l_row = small.tile([128, 512], FP32, tag="lrow")
                r_row = small.tile([128, 512], FP32, tag="rrow")
                nc.vector.tensor_copy(l_row[0:1, :], l_ps[0:1, :])
                nc.vector.reciprocal(r_row[0:1, :], l_row[0:1, :])
                rb_ps = ps_proj_pool.tile([128, 512], FP32, tag="psp")
                nc.tensor.matmul(rb_ps[:], ones_row[:], r_row[0:1, :],
                                 start=True, stop=True)
                rb_sb = small.tile([128, 512], BF16, tag="rb")
                nc.scalar.copy(rb_sb[:], rb_ps[:])
                oT_sb = small.tile([128, 512], BF16, tag="ot")
                nc.vector.tensor_copy(oT_sb[:], out_ps[:])
                nm_sb = small.tile([128, 512], BF16, tag="nm")
                nc.vector.tensor_mul(nm_sb[:], oT_sb[:], rb_sb[:])
                ps4 = ps_t_pool.tile([128, 512], BF16, tag="pst")
                for j in range(4):
                    nc.tensor.transpose(
                        ps4[:, 128 * j:128 * (j + 1)],
                        nm_sb[:, 128 * j:128 * (j + 1)], identb[:])
                ostage = small.tile([128, 512], FP32, tag="os")
                nc.scalar.copy(ostage[:], ps4[:])
                for j in range(4):
                    nc.sync.dma_start(
                        out_ext[512 * c + 128 * j:512 * c + 128 * (j + 1), :],
                        ostage[:, 128 * j:128 * (j + 1)])

    nc.compile()
    return nc


_NC_CACHE = None


def _get_nc():
    global _NC_CACHE
    if _NC_CACHE is None:
        _NC_CACHE = build()
    return _NC_CACHE


def kernel(x, Wq, Wk, Wv):
    """x: [8, 2048, 1024] f32; Wq/Wk/Wv: [1024, 128] f32 -> [8, 2048, 128] f32."""
    x = np.ascontiguousarray(x, dtype=np.float32)
    Wq = np.ascontiguousarray(Wq, dtype=np.float32)
    Wk = np.ascontiguousarray(Wk, dtype=np.float32)
    Wv = np.ascontiguousarray(Wv, dtype=np.float32)
    B = x.shape[0]
    assert x.shape == (B, T, E) and B == 8

    nc = _get_nc()
    in_maps = [{"x": x[b], "Wq": Wq, "Wk": Wk, "Wv": Wv} for b in range(B)]
    res = run_bass_kernel_spmd(nc, in_maps, core_ids=list(range(B)))
    return np.stack([res.results[b]["out"] for b in range(B)], axis=0)


if __name__ == "__main__":
    rng = np.random.default_rng(0)
    x = rng.standard_normal((8, T, E), dtype=np.float32)
    s = 1.0 / np.sqrt(E)
    Wq = (rng.standard_normal((E, H)) * s).astype(np.float32)
    Wk = (rng.standard_normal((E, H)) * s).astype(np.float32)
    Wv = (rng.standard_normal((E, H)) * s).astype(np.float32)
    out = kernel(x=x, Wq=Wq, Wk=Wk, Wv=Wv)
    print("out", out.shape, out.dtype, np.abs(out).max())


# revision 38
# speedup vs baseline: 1.1846x; 1.0048x over previous
"""Single-head causal attention (nanoGPT Head) on 8 TRN2 NeuronCores.

Sharding: data-parallel over batch. B=8 batch elements -> one per core.
Each core computes, for its x_b [T=2048, E=1024] and shared Wq/Wk/Wv [E, H=128]:
    q = x @ Wq ; k = x @ Wk ; v = x @ Wv
    out = softmax(causal(q k^T / sqrt(H))) v          -> [T, H]

Per-core pipeline (all matmuls contract along the SBUF partition dim):
  1. DMA x tiles [128, E]; PE-transpose (f32) into xT [E-part, T] rounded to
     f32r during the mandatory PSUM->SBUF copy (f32r streams at 1 cyc/row).
  2. qT/kT = W^T x^T via f32r matmuls (N=512 chunks, 8 e-tile accumulation);
     vT likewise, evacuated as bf16 and re-transposed to V [kv-part, H].
  3. Per q-tile (128 queries): S chunks [128, <=512] = qT_tile^T kT (f32r);
     causal tri-mask added on the diagonal 128-col block in PSUM; ACT Exp
     with accum_out produces P (bf16) + exact row sums l. No max-subtraction:
     scores are ~N(0,1), bounded ~ +-6, exp is safe in f32.
  4. PE-transpose P tiles (bf16) -> PT; PV matmuls (bf16, N=128) accumulate
     out [q, H] in PSUM over kv tiles; multiply by 1/l (per-partition) on the
     PSUM->SBUF copy; DMA out.
"""
import numpy as np

import concourse.bacc as bacc
import concourse.mybir as mybir
import concourse.tile as tile
from concourse.bass_utils import run_bass_kernel_spmd
from concourse.masks import make_identity, make_causal_mask

FP32 = mybir.dt.float32
FP32R = mybir.dt.float32r
BF16 = mybir.dt.bfloat16
AF = mybir.ActivationFunctionType

T = 2048          # sequence length (per core)
E = 1024          # embedding dim
H = 128           # head size
NT = T // 128     # 16 query/kv tiles
NE = E // 128     # 8 embedding tiles
SCALE = 1.0 / float(np.sqrt(H))
MASK_VAL = -1e9


def build():
    nc = bacc.Bacc()
    x_ext = nc.declare_dram_parameter("x", [T, E], FP32, isOutput=False)
    wq_ext = nc.declare_dram_parameter("Wq", [E, H], FP32, isOutput=False)
    wk_ext = nc.declare_dram_parameter("Wk", [E, H], FP32, isOutput=False)
    wv_ext = nc.declare_dram_parameter("Wv", [E, H], FP32, isOutput=False)
    out_ext = nc.declare_dram_parameter("out", [T, H], FP32, isOutput=True)

    with tile.TileContext(nc) as tc:
        with (
            tc.tile_pool(name="const", bufs=1) as const,
            tc.tile_pool(name="big", bufs=1) as big,
            tc.tile_pool(name="xstage", bufs=3) as xstage,
            tc.tile_pool(name="pbuf", bufs=3) as pbuf,
            tc.tile_pool(name="ptbuf", bufs=3) as ptbuf,
            tc.tile_pool(name="small", bufs=3) as small,
            tc.tile_pool(name="ps_t", bufs=2, space="PSUM") as ps_t_pool,
            tc.tile_pool(name="ps_proj", bufs=2, space="PSUM") as ps_proj_pool,
            tc.tile_pool(name="ps_s", bufs=3, space="PSUM") as ps_s_pool,
            tc.tile_pool(name="ps_l", bufs=1, space="PSUM") as ps_l_pool,
        ):
            # ---- constants (built on-chip, no DMA waits) ----
            ident = const.tile([128, 128], FP32, tag="ident")
            identb = const.tile([128, 128], BF16, tag="identb")
            mask_tri = const.tile([128, 128], FP32, tag="mask")
            make_identity(nc, ident[:])
            make_identity(nc, identb[:])
            make_causal_mask(nc, mask_tri[:], mask_val=MASK_VAL)

            # ---- weights: DMA f32, round to f32r ----
            w_r = []
            for name, ext in (("wq", wq_ext), ("wk", wk_ext), ("wv", wv_ext)):
                w_f = const.tile([128, E], FP32, tag=f"{name}f")
                # W[(k p) h] -> sbuf[p, (k h)]
                nc.gpsimd.dma_start(
                    w_f[:].rearrange("p (k h) -> p k h", k=NE),
                    ext[:].rearrange("(k p) h -> p k h", p=128))
                w_rr = const.tile([128, E], FP32R, tag=f"{name}r")
                nc.gpsimd.tensor_copy(w_rr[:], w_f[:])
                w_r.append(w_rr)
            wq_r, wk_r, wv_r = w_r

            # ---- persistent big buffers ----
            xT = big.tile([128, NE * T], FP32R, tag="xT")       # [e-part, k*T + t]
            qT = big.tile([128, T], FP32R, tag="qT")            # [h, t]
            kT = big.tile([128, T], FP32R, tag="kT")            # [h, t]
            vT = big.tile([128, T], BF16, tag="vT")             # [h, t]
            V = big.tile([128, T], BF16, tag="V")               # [kv-part, j*H + h]

            # ---- phase 1: x -> xT (PE transpose, 2 batched copies per tile) ----
            for i in range(NT):
                x_t = xstage.tile([128, E], FP32, tag="xs")
                nc.sync.dma_start(x_t[:], x_ext[128 * i:128 * (i + 1), :])
                for g in range(2):                       # groups of 4 e-tiles
                    ps4 = ps_t_pool.tile([128, 512], FP32, tag="pst")
                    for kk in range(4):
                        k = 4 * g + kk
                        nc.tensor.transpose(
                            ps4[:, 128 * kk:128 * (kk + 1)],
                            x_t[:, 128 * k:128 * (k + 1)], ident[:])
                    # scatter the 4 transposed blocks to their e-tile columns
                    dst = xT[:].rearrange("p (k t) -> p k t", k=NE)[
                        :, 4 * g:4 * (g + 1), 128 * i:128 * (i + 1)]
                    eng = nc.vector if (i + g) % 2 == 0 else nc.scalar
                    if eng is nc.vector:
                        nc.vector.tensor_copy(dst, ps4[:].rearrange("p (k t) -> p k t", k=4))
                    else:
                        nc.scalar.copy(dst, ps4[:].rearrange("p (k t) -> p k t", k=4))

            # ---- phase 2: projections qT/kT/vT (f32r, N=512) ----
            for c in range(T // 512):
                sl = slice(512 * c, 512 * (c + 1))
                for pi, (w, dstT) in enumerate(((wq_r, qT), (wk_r, kT), (wv_r, vT))):
                    psp = ps_proj_pool.tile([128, 512], FP32, tag="psp")
                    for k in range(NE):
                        nc.tensor.matmul(
                            psp[:], w[:, 128 * k:128 * (k + 1)],
                            xT[:, k * T + 512 * c:k * T + 512 * (c + 1)],
                            start=(k == 0), stop=(k == NE - 1))
                    if pi == 0:
                        nc.scalar.copy(dstT[:, sl], psp[:])
                    elif pi == 1:
                        nc.vector.tensor_copy(dstT[:, sl], psp[:])
                    else:
                        nc.vector.tensor_copy(dstT[:, sl], psp[:])  # vT as bf16

            # ---- phase 3: V = vT^T  (bf16 transposes) ----
            for g in range(4):
                ps4 = ps_t_pool.tile([128, 512], BF16, tag="pst")
                for jj in range(4):
                    j = 4 * g + jj
                    nc.tensor.transpose(
                        ps4[:, 128 * jj:128 * (jj + 1)],
                        vT[:, 128 * j:128 * (j + 1)], identb[:])
                eng = nc.vector if g % 2 == 0 else nc.scalar
                if eng is nc.vector:
                    nc.vector.tensor_copy(V[:, 512 * g:512 * (g + 1)], ps4[:])
                else:
                    nc.scalar.copy(V[:, 512 * g:512 * (g + 1)], ps4[:])

            # ---- phase 4: transposed-score attention, software-pipelined.
            # Per q-chunk c (512 queries), kv-tile t: S^T[kv,q] = kT_t^T qT_c
            # (f32r N=512); mask on diagonal tiles; ACT Exp -> P^T bf16 (no P
            # transposes); l row via ones-matmul (M=1); out^T += V_t^T P^T
            # (N=512). exp(t) overlaps S(t+1) + l/PV(t-1) on the in-order PE
            # queue. Finalize: 1/l broadcast via K=1 outer product, normalize
            # in [h,q], PE-transpose to [q,h], DMA out.
            ones_bf = const.tile([128, 1], BF16, tag="ones")
            nc.gpsimd.memset(ones_bf[:], 1.0)
            ones_row = const.tile([1, 128], FP32, tag="onesr")
            nc.gpsimd.memset(ones_row[:], 1.0)
            mask_triT = const.tile([128, 128], FP32, tag="maskT")
            nc.gpsimd.memset(mask_triT[:], 0.0)
            nc.gpsimd.affine_select(
                out=mask_triT[:], in_=mask_triT[:],
                compare_op=mybir.AluOpType.is_ge, fill=MASK_VAL, base=0,
                # keep 0.0 where q - kv >= 0; fill below the diagonal
                pattern=[[1, 128]], channel_multiplier=-1)

            fin_state = {}

            def kv_chunk(c):
                ncv = 4 * c + 4
                out_ps = ps_proj_pool.tile([128, 512], FP32, tag="psp")
                l_ps = ps_l_pool.tile([128, 512], FP32, tag="psl")
                pbs = {}

                def s_exp(t):
                    pss = ps_s_pool.tile([128, 512], FP32, tag="pss")
                    nc.tensor.matmul(
                        pss[:], kT[:, 128 * t:128 * (t + 1)],
                        qT[:, 512 * c:512 * (c + 1)], start=True, stop=True)
                    d = t - 4 * c
                    off = 0
                    if d >= 0:                         # diagonal tile
                        nc.vector.tensor_add(
                            pss[:, 128 * d:128 * (d + 1)],
                            pss[:, 128 * d:128 * (d + 1)], mask_triT[:])
                        off = 128 * d                  # cols < off fully invalid
                    pb = ptbuf.tile([128, 512], BF16, tag="pt")
                    if off > 0:
                        nc.gpsimd.memset(pb[:, :off], 0.0)
                    nc.scalar.activation(
                        pb[:, off:], pss[:, off:], AF.Exp, bias=0.0, scale=SCALE)
                    pbs[t] = pb

                def acc(t):
                    pb = pbs.pop(t)
                    nc.tensor.matmul(l_ps[0:1, :], ones_bf[:], pb[:],
                                     start=(t == 0), stop=(t == ncv - 1))
                    nc.tensor.matmul(out_ps[:], V[:, 128 * t:128 * (t + 1)], pb[:],
                                     start=(t == 0), stop=(t == ncv - 1))

                for t in range(ncv):
                    s_exp(t)
                    if t >= 1:
                        acc(t - 1)
                acc(ncv - 1)
                # evacuate accumulators now (frees the psp slot for the next
                # chunk) and compute 1/l; heavy finalize deferred one chunk
                l_row = small.tile([128, 512], FP32, tag="lrow")
                r_row = small.tile([128, 512], FP32, tag="rrow")
                nc.vector.tensor_copy(l_row[0:1, :], l_ps[0:1, :])
                nc.vector.reciprocal(r_row[0:1, :], l_row[0:1, :])
                oT_sb = small.tile([128, 512], BF16, tag="ot")
                nc.vector.tensor_copy(oT_sb[:], out_ps[:])
                fin_state[c] = (r_row, oT_sb)

            def finalize(c):
                r_row, oT_sb = fin_state.pop(c)
                rb_ps = ps_proj_pool.tile([128, 512], FP32, tag="psp")
                nc.tensor.matmul(rb_ps[:], ones_row[:], r_row[0:1, :],
                                 start=True, stop=True)
                rb_sb = small.tile([128, 512], BF16, tag="rb")
                nc.scalar.copy(rb_sb[:], rb_ps[:])
                nm_sb = small.tile([128, 512], BF16, tag="nm")
                nc.vector.tensor_mul(nm_sb[:], oT_sb[:], rb_sb[:])
                ps4 = ps_t_pool.tile([128, 512], BF16, tag="pst")
                for j in range(4):
                    nc.tensor.transpose(
                        ps4[:, 128 * j:128 * (j + 1)],
                        nm_sb[:, 128 * j:128 * (j + 1)], identb[:])
                ostage = small.tile([128, 512], FP32, tag="os")
                nc.scalar.copy(ostage[:], ps4[:])
                for j in range(4):
                    nc.sync.dma_start(
                        out_ext[512 * c + 128 * j:512 * c + 128 * (j + 1), :],
                        ostage[:, 128 * j:128 * (j + 1)])

            for idx, c in enumerate((3, 2, 1, 0)):
                kv_chunk(c)
                if idx >= 1:
                    finalize((3, 2, 1, 0)[idx - 1])
            finalize(0)

    nc.compile()
    return nc


_NC_CACHE = None


def _get_nc():
    global _NC_CACHE
    if _NC_CACHE is None:
        _NC_CACHE = build()
    return _NC_CACHE


def kernel(x, Wq, Wk, Wv):
    """x: [8, 2048, 1024] f32; Wq/Wk/Wv: [1024, 128] f32 -> [8, 2048, 128] f32."""
    x = np.ascontiguousarray(x, dtype=np.float32)
    Wq = np.ascontiguousarray(Wq, dtype=np.float32)
    Wk = np.ascontiguousarray(Wk, dtype=np.float32)
    Wv = np.ascontiguousarray(Wv, dtype=np.float32)
    B = x.shape[0]
    assert x.shape == (B, T, E) and B == 8

    nc = _get_nc()
    in_maps = [{"x": x[b], "Wq": Wq, "Wk": Wk, "Wv": Wv} for b in range(B)]
    res = run_bass_kernel_spmd(nc, in_maps, core_ids=list(range(B)))
    return np.stack([res.results[b]["out"] for b in range(B)], axis=0)


if __name__ == "__main__":
    rng = np.random.default_rng(0)
    x = rng.standard_normal((8, T, E), dtype=np.float32)
    s = 1.0 / np.sqrt(E)
    Wq = (rng.standard_normal((E, H)) * s).astype(np.float32)
    Wk = (rng.standard_normal((E, H)) * s).astype(np.float32)
    Wv = (rng.standard_normal((E, H)) * s).astype(np.float32)
    out = kernel(x=x, Wq=Wq, Wk=Wk, Wv=Wv)
    print("out", out.shape, out.dtype, np.abs(out).max())


# revision 40
# speedup vs baseline: 1.4641x; 1.2359x over previous
"""Single-head causal attention (nanoGPT Head) on 8 TRN2 NeuronCores.

Sharding: data-parallel over batch. B=8 batch elements -> one per core.
Each core computes, for its x_b [T=2048, E=1024] and shared Wq/Wk/Wv [E, H=128]:
    q = x @ Wq ; k = x @ Wk ; v = x @ Wv
    out = softmax(causal(q k^T / sqrt(H))) v          -> [T, H]

Per-core pipeline (all matmuls contract along the SBUF partition dim):
  1. DMA x tiles [128, E]; PE-transpose (f32) into xT [E-part, T] rounded to
     f32r during the mandatory PSUM->SBUF copy (f32r streams at 1 cyc/row).
  2. qT/kT = W^T x^T via f32r matmuls (N=512 chunks, 8 e-tile accumulation);
     vT likewise, evacuated as bf16 and re-transposed to V [kv-part, H].
  3. Per q-tile (128 queries): S chunks [128, <=512] = qT_tile^T kT (f32r);
     causal tri-mask added on the diagonal 128-col block in PSUM; ACT Exp
     with accum_out produces P (bf16) + exact row sums l. No max-subtraction:
     scores are ~N(0,1), bounded ~ +-6, exp is safe in f32.
  4. PE-transpose P tiles (bf16) -> PT; PV matmuls (bf16, N=128) accumulate
     out [q, H] in PSUM over kv tiles; multiply by 1/l (per-partition) on the
     PSUM->SBUF copy; DMA out.
"""
import numpy as np

import concourse.bacc as bacc
import concourse.mybir as mybir
import concourse.tile as tile
from concourse.bass_utils import run_bass_kernel_spmd
from concourse.masks import make_identity, make_causal_mask

FP32 = mybir.dt.float32
FP32R = mybir.dt.float32r
BF16 = mybir.dt.bfloat16
AF = mybir.ActivationFunctionType

T = 2048          # sequence length (per core)
E = 1024          # embedding dim
H = 128           # head size
NT = T // 128     # 16 query/kv tiles
NE = E // 128     # 8 embedding tiles
SCALE = 1.0 / float(np.sqrt(H))
MASK_VAL = -1e9


def build():
    nc = bacc.Bacc()
    x_ext = nc.declare_dram_parameter("x", [T, E], FP32, isOutput=False)
    wq_ext = nc.declare_dram_parameter("Wq", [E, H], FP32, isOutput=False)
    wk_ext = nc.declare_dram_parameter("Wk", [E, H], FP32, isOutput=False)
    wv_ext = nc.declare_dram_parameter("Wv", [E, H], FP32, isOutput=False)
    out_ext = nc.declare_dram_parameter("out", [T, H], FP32, isOutput=True)

    with tile.TileContext(nc) as tc:
        with (
            tc.tile_pool(name="const", bufs=1) as const,
            tc.tile_pool(name="big", bufs=1) as big,
            tc.tile_pool(name="xstage", bufs=4) as xstage,
            tc.tile_pool(name="pbuf", bufs=3) as pbuf,
            tc.tile_pool(name="ptbuf", bufs=4) as ptbuf,
            tc.tile_pool(name="small", bufs=3) as small,
            tc.tile_pool(name="ps_t", bufs=2, space="PSUM") as ps_t_pool,
            tc.tile_pool(name="ps_proj", bufs=2, space="PSUM") as ps_proj_pool,
            tc.tile_pool(name="ps_s", bufs=4, space="PSUM") as ps_s_pool,
        ):
            # ---- constants (built on-chip, no DMA waits) ----
            ident = const.tile([128, 128], FP32, tag="ident")
            identb = const.tile([128, 128], BF16, tag="identb")
            mask_tri = const.tile([128, 128], FP32, tag="mask")
            make_identity(nc, ident[:])
            make_identity(nc, identb[:])
            make_causal_mask(nc, mask_tri[:], mask_val=MASK_VAL)

            # ---- weights: DMA f32, round to f32r ----
            w_r = []
            for name, ext in (("wq", wq_ext), ("wk", wk_ext), ("wv", wv_ext)):
                w_f = const.tile([128, E], FP32, tag=f"{name}f")
                # W[(k p) h] -> sbuf[p, (k h)]
                nc.gpsimd.dma_start(
                    w_f[:].rearrange("p (k h) -> p k h", k=NE),
                    ext[:].rearrange("(k p) h -> p k h", p=128))
                w_rr = const.tile([128, E], FP32R, tag=f"{name}r")
                nc.gpsimd.tensor_copy(w_rr[:], w_f[:])
                w_r.append(w_rr)
            wq_r, wk_r, wv_r = w_r

            # ---- persistent big buffers ----
            xT = big.tile([128, NE * T], FP32R, tag="xT")       # [e-part, k*T + t]
            qT = big.tile([128, T], FP32R, tag="qT")            # [h, t]
            kT = big.tile([128, T], FP32R, tag="kT")            # [h, t]
            vT = big.tile([128, T], BF16, tag="vT")             # [h, t]
            V = big.tile([128, T], BF16, tag="V")               # [kv-part, j*H + h]

            # ---- phase 1: x -> xT (PE transpose, 2 batched copies per tile) ----
            for i in range(NT):
                x_t = xstage.tile([128, E], FP32, tag="xs")
                nc.sync.dma_start(x_t[:], x_ext[128 * i:128 * (i + 1), :])
                for g in range(2):                       # groups of 4 e-tiles
                    ps4 = ps_t_pool.tile([128, 512], FP32, tag="pst")
                    for kk in range(4):
                        k = 4 * g + kk
                        nc.tensor.transpose(
                            ps4[:, 128 * kk:128 * (kk + 1)],
                            x_t[:, 128 * k:128 * (k + 1)], ident[:])
                    # scatter the 4 transposed blocks to their e-tile columns
                    dst = xT[:].rearrange("p (k t) -> p k t", k=NE)[
                        :, 4 * g:4 * (g + 1), 128 * i:128 * (i + 1)]
                    eng = nc.vector if (i + g) % 2 == 0 else nc.scalar
                    if eng is nc.vector:
                        nc.vector.tensor_copy(dst, ps4[:].rearrange("p (k t) -> p k t", k=4))
                    else:
                        nc.scalar.copy(dst, ps4[:].rearrange("p (k t) -> p k t", k=4))

            # ---- phase 2: projections qT/kT/vT (f32r, N=512) ----
            for c in range(T // 512):
                sl = slice(512 * c, 512 * (c + 1))
                for pi, (w, dstT) in enumerate(((wq_r, qT), (wk_r, kT), (wv_r, vT))):
                    psp = ps_proj_pool.tile([128, 512], FP32, tag="psp")
                    for k in range(NE):
                        nc.tensor.matmul(
                            psp[:], w[:, 128 * k:128 * (k + 1)],
                            xT[:, k * T + 512 * c:k * T + 512 * (c + 1)],
                            start=(k == 0), stop=(k == NE - 1))
                    if pi == 0:
                        nc.scalar.copy(dstT[:, sl], psp[:])
                    elif pi == 1:
                        nc.vector.tensor_copy(dstT[:, sl], psp[:])
                    else:
                        nc.vector.tensor_copy(dstT[:, sl], psp[:])  # vT as bf16

            # ---- phase 3: V = vT^T  (bf16 transposes) ----
            for g in range(4):
                ps4 = ps_t_pool.tile([128, 512], BF16, tag="pst")
                for jj in range(4):
                    j = 4 * g + jj
                    nc.tensor.transpose(
                        ps4[:, 128 * jj:128 * (jj + 1)],
                        vT[:, 128 * j:128 * (j + 1)], identb[:])
                eng = nc.vector if g % 2 == 0 else nc.scalar
                if eng is nc.vector:
                    nc.vector.tensor_copy(V[:, 512 * g:512 * (g + 1)], ps4[:])
                else:
                    nc.scalar.copy(V[:, 512 * g:512 * (g + 1)], ps4[:])

            # ---- phase 4: attention, software-pipelined one q-tile deep:
            # exp(qi) on ACT overlaps PT/PV(prev) on the in-order PE queue.
            # q-tiles processed largest-first so the serial tail is the
            # smallest tile. ----
            state = {}

            def attn_S(qi):
                nkv = qi + 1
                kv_len = 128 * nkv
                nchunks = (kv_len + 511) // 512

                P = pbuf.tile([128, T], BF16, tag="P")
                l_parts = small.tile([128, 4], FP32, tag="lp")
                # S chunks + mask + exp
                for j in range(nchunks):
                    valid = min(512, kv_len - 512 * j)
                    n = max(valid, 256)              # f32r needs N>=256 for 1 cyc/row
                    pss = ps_s_pool.tile([128, 512], FP32, tag="pss")
                    nc.tensor.matmul(
                        pss[:, :n], qT[:, 128 * qi:128 * (qi + 1)],
                        kT[:, 512 * j:512 * j + n],
                        start=True, stop=True)
                    if qi // 4 == j:                 # diagonal 128-block lives here
                        off = 128 * (qi % 4)
                        nc.vector.tensor_add(
                            pss[:, off:off + 128], pss[:, off:off + 128], mask_tri[:])
                    nc.scalar.activation(
                        P[:, 512 * j:512 * j + valid], pss[:, :valid], AF.Exp,
                        bias=0.0, scale=SCALE, accum_out=l_parts[:, j:j + 1])

                l_sum = small.tile([128, 1], FP32, tag="ls")
                recip = small.tile([128, 1], FP32, tag="rc")
                nc.vector.reduce_sum(l_sum[:], l_parts[:, :nchunks],
                                     axis=mybir.AxisListType.X)
                nc.vector.reciprocal(recip[:], l_sum[:])
                state[qi] = (P, recip)

            def attn_PV(qi):
                nkv = qi + 1
                P, recip = state.pop(qi)
                # P^T tiles (batched in groups of 4) + PV accumulation
                pso = ps_proj_pool.tile([128, 128], FP32, tag="psp")
                for g in range((nkv + 3) // 4):
                    cnt = min(4, nkv - 4 * g)
                    ps4 = ps_t_pool.tile([128, 512], BF16, tag="pst")
                    for jj in range(cnt):
                        j = 4 * g + jj
                        nc.tensor.transpose(
                            ps4[:, 128 * jj:128 * (jj + 1)],
                            P[:, 128 * j:128 * (j + 1)], identb[:])
                    pt = ptbuf.tile([128, 512], BF16, tag="pt")
                    eng = nc.vector if g % 2 == 0 else nc.scalar
                    if eng is nc.vector:
                        nc.vector.tensor_copy(pt[:, :128 * cnt], ps4[:, :128 * cnt])
                    else:
                        nc.scalar.copy(pt[:, :128 * cnt], ps4[:, :128 * cnt])
                    for jj in range(cnt):
                        j = 4 * g + jj
                        nc.tensor.matmul(
                            pso[:], pt[:, 128 * jj:128 * (jj + 1)],
                            V[:, 128 * j:128 * (j + 1)],
                            start=(j == 0), stop=(j == nkv - 1))

                out_sb = small.tile([128, H], FP32, tag="os")
                nc.vector.tensor_scalar_mul(out_sb[:], pso[:], recip[:])
                # two half-DMAs on separate queues halve the post-PV tail
                nc.sync.dma_start(out_ext[128 * qi:128 * qi + 64, :], out_sb[:64, :])
                nc.sync.dma_start(out_ext[128 * qi + 64:128 * (qi + 1), :], out_sb[64:, :])

            order = list(range(NT - 1, -1, -1))       # largest q-tile first
            for idx, qi in enumerate(order):
                attn_S(qi)
                if idx >= 1:
                    attn_PV(order[idx - 1])
            attn_PV(order[-1])

    nc.compile()
    return nc


_NC_CACHE = None


def _get_nc():
    global _NC_CACHE
    if _NC_CACHE is None:
        _NC_CACHE = build()
    return _NC_CACHE


def kernel(x, Wq, Wk, Wv):
    """x: [8, 2048, 1024] f32; Wq/Wk/Wv: [1024, 128] f32 -> [8, 2048, 128] f32."""
    x = np.ascontiguousarray(x, dtype=np.float32)
    Wq = np.ascontiguousarray(Wq, dtype=np.float32)
    Wk = np.ascontiguousarray(Wk, dtype=np.float32)
    Wv = np.ascontiguousarray(Wv, dtype=np.float32)
    B = x.shape[0]
    assert x.shape == (B, T, E) and B == 8

    nc = _get_nc()
    in_maps = [{"x": x[b], "Wq": Wq, "Wk": Wk, "Wv": Wv} for b in range(B)]
    res = run_bass_kernel_spmd(nc, in_maps, core_ids=list(range(B)))
    return np.stack([res.results[b]["out"] for b in range(B)], axis=0)


if __name__ == "__main__":
    rng = np.random.default_rng(0)
    x = rng.standard_normal((8, T, E), dtype=np.float32)
    s = 1.0 / np.sqrt(E)
    Wq = (rng.standard_normal((E, H)) * s).astype(np.float32)
    Wk = (rng.standard_normal((E, H)) * s).astype(np.float32)
    Wv = (rng.standard_normal((E, H)) * s).astype(np.float32)
    out = kernel(x=x, Wq=Wq, Wk=Wk, Wv=Wv)
    print("out", out.shape, out.dtype, np.abs(out).max())


# revision 41
# speedup vs baseline: 1.4700x; 1.0041x over previous
"""Single-head causal attention (nanoGPT Head) on 8 TRN2 NeuronCores.

Sharding: data-parallel over batch. B=8 batch elements -> one per core.
Each core computes, for its x_b [T=2048, E=1024] and shared Wq/Wk/Wv [E, H=128]:
    q = x @ Wq ; k = x @ Wk ; v = x @ Wv
    out = softmax(causal(q k^T / sqrt(H))) v          -> [T, H]

Per-core pipeline (all matmuls contract along the SBUF partition dim):
  1. DMA x tiles [128, E]; PE-transpose (f32) into xT [E-part, T] rounded to
     f32r during the mandatory PSUM->SBUF copy (f32r streams at 1 cyc/row).
  2. qT/kT = W^T x^T via f32r matmuls (N=512 chunks, 8 e-tile accumulation);
     vT likewise, evacuated as bf16 and re-transposed to V [kv-part, H].
  3. Per q-tile (128 queries): S chunks [128, <=512] = qT_tile^T kT (f32r);
     causal tri-mask added on the diagonal 128-col block in PSUM; ACT Exp
     with accum_out produces P (bf16) + exact row sums l. No max-subtraction:
     scores are ~N(0,1), bounded ~ +-6, exp is safe in f32.
  4. PE-transpose P tiles (bf16) -> PT; PV matmuls (bf16, N=128) accumulate
     out [q, H] in PSUM over kv tiles; multiply by 1/l (per-partition) on the
     PSUM->SBUF copy; DMA out.
"""
import numpy as np

import concourse.bacc as bacc
import concourse.mybir as mybir
import concourse.tile as tile
from concourse.bass_utils import run_bass_kernel_spmd
from concourse.masks import make_identity, make_causal_mask

FP32 = mybir.dt.float32
FP32R = mybir.dt.float32r
BF16 = mybir.dt.bfloat16
AF = mybir.ActivationFunctionType

T = 2048          # sequence length (per core)
E = 1024          # embedding dim
H = 128           # head size
NT = T // 128     # 16 query/kv tiles
NE = E // 128     # 8 embedding tiles
SCALE = 1.0 / float(np.sqrt(H))
MASK_VAL = -1e9


def build():
    nc = bacc.Bacc()
    x_ext = nc.declare_dram_parameter("x", [T, E], FP32, isOutput=False)
    wq_ext = nc.declare_dram_parameter("Wq", [E, H], FP32, isOutput=False)
    wk_ext = nc.declare_dram_parameter("Wk", [E, H], FP32, isOutput=False)
    wv_ext = nc.declare_dram_parameter("Wv", [E, H], FP32, isOutput=False)
    out_ext = nc.declare_dram_parameter("out", [T, H], FP32, isOutput=True)

    with tile.TileContext(nc) as tc:
        with (
            tc.tile_pool(name="const", bufs=1) as const,
            tc.tile_pool(name="big", bufs=1) as big,
            tc.tile_pool(name="xstage", bufs=5) as xstage,
            tc.tile_pool(name="pbuf", bufs=4) as pbuf,
            tc.tile_pool(name="ptbuf", bufs=4) as ptbuf,
            tc.tile_pool(name="small", bufs=4) as small,
            tc.tile_pool(name="ps_t", bufs=2, space="PSUM") as ps_t_pool,
            tc.tile_pool(name="ps_proj", bufs=2, space="PSUM") as ps_proj_pool,
            tc.tile_pool(name="ps_s", bufs=4, space="PSUM") as ps_s_pool,
        ):
            # ---- constants (built on-chip, no DMA waits) ----
            ident = const.tile([128, 128], FP32, tag="ident")
            identb = const.tile([128, 128], BF16, tag="identb")
            mask_tri = const.tile([128, 128], FP32, tag="mask")
            make_identity(nc, ident[:])
            make_identity(nc, identb[:])
            make_causal_mask(nc, mask_tri[:], mask_val=MASK_VAL)

            # ---- weights: DMA f32, round to f32r ----
            w_r = []
            for name, ext in (("wq", wq_ext), ("wk", wk_ext), ("wv", wv_ext)):
                w_f = const.tile([128, E], FP32, tag=f"{name}f")
                # W[(k p) h] -> sbuf[p, (k h)]
                nc.gpsimd.dma_start(
                    w_f[:].rearrange("p (k h) -> p k h", k=NE),
                    ext[:].rearrange("(k p) h -> p k h", p=128))
                w_rr = const.tile([128, E], FP32R, tag=f"{name}r")
                nc.gpsimd.tensor_copy(w_rr[:], w_f[:])
                w_r.append(w_rr)
            wq_r, wk_r, wv_r = w_r

            # ---- persistent big buffers ----
            xT = big.tile([128, NE * T], FP32R, tag="xT")       # [e-part, k*T + t]
            qT = big.tile([128, T], FP32R, tag="qT")            # [h, t]
            kT = big.tile([128, T], FP32R, tag="kT")            # [h, t]
            vT = big.tile([128, T], BF16, tag="vT")             # [h, t]
            V = big.tile([128, T], BF16, tag="V")               # [kv-part, j*H + h]

            # ---- phase 1: x -> xT (PE transpose, 2 batched copies per tile) ----
            for i in range(NT):
                x_t = xstage.tile([128, E], FP32, tag="xs")
                nc.sync.dma_start(x_t[:], x_ext[128 * i:128 * (i + 1), :])
                for g in range(2):                       # groups of 4 e-tiles
                    ps4 = ps_t_pool.tile([128, 512], FP32, tag="pst")
                    for kk in range(4):
                        k = 4 * g + kk
                        nc.tensor.transpose(
                            ps4[:, 128 * kk:128 * (kk + 1)],
                            x_t[:, 128 * k:128 * (k + 1)], ident[:])
                    # scatter the 4 transposed blocks to their e-tile columns
                    dst = xT[:].rearrange("p (k t) -> p k t", k=NE)[
                        :, 4 * g:4 * (g + 1), 128 * i:128 * (i + 1)]
                    eng = nc.vector if (i + g) % 2 == 0 else nc.scalar
                    if eng is nc.vector:
                        nc.vector.tensor_copy(dst, ps4[:].rearrange("p (k t) -> p k t", k=4))
                    else:
                        nc.scalar.copy(dst, ps4[:].rearrange("p (k t) -> p k t", k=4))

            # ---- phase 2: projections qT/kT/vT (f32r, N=512) ----
            for c in range(T // 512):
                sl = slice(512 * c, 512 * (c + 1))
                for pi, (w, dstT) in enumerate(((wq_r, qT), (wk_r, kT), (wv_r, vT))):
                    psp = ps_proj_pool.tile([128, 512], FP32, tag="psp")
                    for k in range(NE):
                        nc.tensor.matmul(
                            psp[:], w[:, 128 * k:128 * (k + 1)],
                            xT[:, k * T + 512 * c:k * T + 512 * (c + 1)],
                            start=(k == 0), stop=(k == NE - 1))
                    if pi == 0:
                        nc.scalar.copy(dstT[:, sl], psp[:])
                    elif pi == 1:
                        nc.vector.tensor_copy(dstT[:, sl], psp[:])
                    else:
                        nc.vector.tensor_copy(dstT[:, sl], psp[:])  # vT as bf16

            # ---- phase 3: V = vT^T  (bf16 transposes) ----
            for g in range(4):
                ps4 = ps_t_pool.tile([128, 512], BF16, tag="pst")
                for jj in range(4):
                    j = 4 * g + jj
                    nc.tensor.transpose(
                        ps4[:, 128 * jj:128 * (jj + 1)],
                        vT[:, 128 * j:128 * (j + 1)], identb[:])
                eng = nc.vector if g % 2 == 0 else nc.scalar
                if eng is nc.vector:
                    nc.vector.tensor_copy(V[:, 512 * g:512 * (g + 1)], ps4[:])
                else:
                    nc.scalar.copy(V[:, 512 * g:512 * (g + 1)], ps4[:])

            # ---- phase 4: attention, software-pipelined one q-tile deep:
            # exp(qi) on ACT overlaps PT/PV(prev) on the in-order PE queue.
            # q-tiles processed largest-first so the serial tail is the
            # smallest tile. ----
            state = {}

            def attn_S(qi):
                nkv = qi + 1
                kv_len = 128 * nkv
                nchunks = (kv_len + 511) // 512

                P = pbuf.tile([128, T], BF16, tag="P")
                l_parts = small.tile([128, 4], FP32, tag="lp")
                # S chunks + mask + exp
                for j in range(nchunks):
                    valid = min(512, kv_len - 512 * j)
                    n = max(valid, 256)              # f32r needs N>=256 for 1 cyc/row
                    pss = ps_s_pool.tile([128, 512], FP32, tag="pss")
                    nc.tensor.matmul(
                        pss[:, :n], qT[:, 128 * qi:128 * (qi + 1)],
                        kT[:, 512 * j:512 * j + n],
                        start=True, stop=True)
                    if qi // 4 == j:                 # diagonal 128-block lives here
                        off = 128 * (qi % 4)
                        nc.vector.tensor_add(
                            pss[:, off:off + 128], pss[:, off:off + 128], mask_tri[:])
                    nc.scalar.activation(
                        P[:, 512 * j:512 * j + valid], pss[:, :valid], AF.Exp,
                        bias=0.0, scale=SCALE, accum_out=l_parts[:, j:j + 1])

                l_sum = small.tile([128, 1], FP32, tag="ls")
                recip = small.tile([128, 1], FP32, tag="rc")
                nc.vector.reduce_sum(l_sum[:], l_parts[:, :nchunks],
                                     axis=mybir.AxisListType.X)
                nc.vector.reciprocal(recip[:], l_sum[:])
                state[qi] = (P, recip)

            def attn_PV(qi):
                nkv = qi + 1
                P, recip = state.pop(qi)
                # P^T tiles (batched in groups of 4) + PV accumulation
                pso = ps_proj_pool.tile([128, 128], FP32, tag="psp")
                for g in range((nkv + 3) // 4):
                    cnt = min(4, nkv - 4 * g)
                    ps4 = ps_t_pool.tile([128, 512], BF16, tag="pst")
                    for jj in range(cnt):
                        j = 4 * g + jj
                        nc.tensor.transpose(
                            ps4[:, 128 * jj:128 * (jj + 1)],
                            P[:, 128 * j:128 * (j + 1)], identb[:])
                    pt = ptbuf.tile([128, 512], BF16, tag="pt")
                    eng = nc.vector if g % 2 == 0 else nc.scalar
                    if eng is nc.vector:
                        nc.vector.tensor_copy(pt[:, :128 * cnt], ps4[:, :128 * cnt])
                    else:
                        nc.scalar.copy(pt[:, :128 * cnt], ps4[:, :128 * cnt])
                    for jj in range(cnt):
                        j = 4 * g + jj
                        nc.tensor.matmul(
                            pso[:], pt[:, 128 * jj:128 * (jj + 1)],
                            V[:, 128 * j:128 * (j + 1)],
                            start=(j == 0), stop=(j == nkv - 1))

                out_sb = small.tile([128, H], FP32, tag="os")
                nc.vector.tensor_scalar_mul(out_sb[:], pso[:], recip[:])
                # two half-DMAs on separate queues halve the post-PV tail
                nc.sync.dma_start(out_ext[128 * qi:128 * qi + 64, :], out_sb[:64, :])
                nc.sync.dma_start(out_ext[128 * qi + 64:128 * (qi + 1), :], out_sb[64:, :])

            order = list(range(NT - 1, -1, -1))       # largest q-tile first
            for idx, qi in enumerate(order):
                attn_S(qi)
                if idx >= 1:
                    attn_PV(order[idx - 1])
            attn_PV(order[-1])

    nc.compile()
    return nc


_NC_CACHE = None


def _get_nc():
    global _NC_CACHE
    if _NC_CACHE is None:
        _NC_CACHE = build()
    return _NC_CACHE


def kernel(x, Wq, Wk, Wv):
    """x: [8, 2048, 1024] f32; Wq/Wk/Wv: [1024, 128] f32 -> [8, 2048, 128] f32."""
    x = np.ascontiguousarray(x, dtype=np.float32)
    Wq = np.ascontiguousarray(Wq, dtype=np.float32)
    Wk = np.ascontiguousarray(Wk, dtype=np.float32)
    Wv = np.ascontiguousarray(Wv, dtype=np.float32)
    B = x.shape[0]
    assert x.shape == (B, T, E) and B == 8

    nc = _get_nc()
    in_maps = [{"x": x[b], "Wq": Wq, "Wk": Wk, "Wv": Wv} for b in range(B)]
    res = run_bass_kernel_spmd(nc, in_maps, core_ids=list(range(B)))
    return np.stack([res.results[b]["out"] for b in range(B)], axis=0)


if __name__ == "__main__":
    rng = np.random.default_rng(0)
    x = rng.standard_normal((8, T, E), dtype=np.float32)
    s = 1.0 / np.sqrt(E)
    Wq = (rng.standard_normal((E, H)) * s).astype(np.float32)
    Wk = (rng.standard_normal((E, H)) * s).astype(np.float32)
    Wv = (rng.standard_normal((E, H)) * s).astype(np.float32)
    out = kernel(x=x, Wq=Wq, Wk=Wk, Wv=Wv)
    print("out", out.shape, out.dtype, np.abs(out).max())
